# revision 2
# baseline (speedup 1.0000x reference)
"""Causal single-head attention (S=8192, d=64) on 8 Trainium2 NeuronCores.

Strategy (sequence-parallel, load-balanced over the causal triangle):
  - Split the sequence into 16 chunks of 512 rows. Core m owns query chunks
    A=m and B=15-m, so every core sees (m+1) + (16-m) = 17 (kv-block, q-chunk)
    pairs of 512x512 scores -- perfectly balanced.
  - Each pair is one "slot". The SPMD program is identical on all cores; the
    per-core schedule is baked into the *input data* (host gathers the slot's
    kv rows and q rows, transposed and bias-augmented).
  - Scores are computed transposed, sT[j, i] (kv j on partitions), so the
    softmax denominator comes from an extra all-ones column of the augmented V
    in the PV matmul, and P^T feeds the PV matmul with no transposes at all.
  - The causal mask is needed only on the two diagonal slots (fixed slot
    positions 0 and 1) and is applied as a post-exp affine_select (zero the
    j > i half), which exactly matches softmax(scores - 1e10*mask).
  - Per-slot partials accumulate into per-chunk SBUF accumulators via a 0/1
    multiplier input (gamma), keeping the program fully static.

Math per slot t with kv block rows Xk = x[512b:512b+512], q rows Xq:
  xkvT = [Xk^T; 1]  (65 x 512)     xqT = [Xq^T; 1]  (65 x 512)
  M    = wk_aug @ wq_aug^T / 8     (65 x 65, on device from weight inputs)
  ktil = M^T @ xkvT                (65 x 512)
  sT   = ktil[:, js]^T @ xqT       (128 x 512 per 128-row j-subchunk, PSUM)
  pT   = exp(sT)                   (masked to causal on diagonal slots)
  oT  += v_aug[js]^T @ pT          (65 x 512; row 64 = softmax denominator)
Final: out[i, :] = oT[0:64, i] / oT[64, i], transposed back via PE transpose.
"""

import sys

sys.path.insert(0, "/opt/trn_rl_repo")

import numpy as np
import concourse.bass as bass
import concourse.mybir as mybir
from concourse import tile
from concourse.bass_utils import run_bass_kernel_spmd

N_CORES = 8
S = 8192
D = 64
CH = 512
NCH = S // CH          # 16 chunks
NSLOT = 17             # (kv, q) pairs per core
JS = 128               # j-subchunk (PSUM partition dim)
NJS = CH // JS         # 4
DA = D + 1             # bias-augmented contraction dim

F32 = mybir.dt.float32
F32R = mybir.dt.float32r

USE_F32R = False       # matmul operand precision: False = full fp32 (4 cyc/row)


def _split_multiwait(nc, max_waits=1):
    """The walrus build in this container accepts only one sync-wait per
    instruction; hoist extra waits onto preceding same-engine NOPs."""
    for func in nc.m.functions:
        for bb in func.blocks:
            new_insts = []
            for inst in bb.instructions:
                si = inst.sync_info
                if si is not None and si.on_wait and len(si.on_wait) > max_waits:
                    waits = list(si.on_wait)
                    rest, head = waits[:-max_waits], waits[-max_waits:]
                    for j, w in enumerate(rest):
                        nop = mybir.InstNoOp(
                            name=f"{inst.name}-wsplit{j}", ins=[], outs=[]
                        )
                        nop.engine = inst.engine
                        nop.sync_info = mybir.SyncInfo(on_wait=[w], on_update=[])
                        new_insts.append(nop)
                    inst.sync_info = mybir.SyncInfo(
                        on_wait=head, on_update=si.on_update
                    )
                new_insts.append(inst)
            bb.instructions = new_insts


def _schedule(m):
    """Slot list [(kv_block, q_chunk)] for core m; diagonal pairs first."""
    A, B = m, NCH - 1 - m
    slots = [(A, A), (B, B)]
    slots += [(b, A) for b in range(A)]
    slots += [(b, B) for b in range(B)]
    gam = [1.0, 0.0] + [1.0] * A + [0.0] * B
    assert len(slots) == NSLOT
    return slots, gam


def _build_program():
    nc = bass.Bass()
    mm_dt = F32R if USE_F32R else F32

    xkvT_d = nc.declare_dram_parameter("xkvT", [DA, NSLOT * CH], F32, isOutput=False)
    xqT_d = nc.declare_dram_parameter("xqT", [DA, NSLOT * CH], F32, isOutput=False)
    wkT_d = nc.declare_dram_parameter("wkT", [D, DA], F32, isOutput=False)
    wqT_d = nc.declare_dram_parameter("wqT", [D, DA], F32, isOutput=False)
    wv_d = nc.declare_dram_parameter("wv_aug", [DA, DA], F32, isOutput=False)
    gam_d = nc.declare_dram_parameter("gam", [DA, NSLOT], F32, isOutput=False)
    ident_d = nc.declare_dram_parameter("ident", [DA, DA], F32, isOutput=False)
    out_d = nc.declare_dram_parameter("out_pair", [2, CH, D], F32, isOutput=True)

    with tile.TileContext(nc) as tc:
        with (
            tc.tile_pool(name="consts", bufs=1) as consts,
            tc.tile_pool(name="acc", bufs=1) as accp,
            tc.tile_pool(name="slot_in", bufs=3) as slot_in,
            tc.tile_pool(name="slot_mid", bufs=3) as slot_mid,
            tc.tile_pool(name="pt", bufs=2) as ptp,
            tc.tile_pool(name="gd", bufs=3) as gdp,
            tc.tile_pool(name="fin", bufs=2) as finp,
            tc.tile_pool(name="ps_s", bufs=1, space="PSUM") as ps_s_p,
            tc.tile_pool(name="ps_o", bufs=2, space="PSUM") as ps_o_p,
            tc.tile_pool(name="ps_k", bufs=1, space="PSUM") as ps_k_p,
            tc.tile_pool(name="ps_v", bufs=1, space="PSUM") as ps_v_p,
        ):
            # ---- constants ----
            wkT = consts.tile([D, DA], F32)
            wqT = consts.tile([D, DA], F32)
            wv = consts.tile([DA, DA], F32)
            gam = consts.tile([DA, NSLOT], F32)
            ident = consts.tile([DA, DA], F32)
            nc.sync.dma_start(out=wkT[:], in_=wkT_d[:])
            nc.sync.dma_start(out=wqT[:], in_=wqT_d[:])
            nc.sync.dma_start(out=wv[:], in_=wv_d[:])
            nc.sync.dma_start(out=gam[:], in_=gam_d[:])
            nc.sync.dma_start(out=ident[:], in_=ident_d[:])

            # M = wk_aug @ wq_aug^T  (scale folded into wq on host)
            ps_m = ps_k_p.tile([DA, DA], F32, tag="psk")
            nc.tensor.matmul(ps_m[:], wkT[:], wqT[:], start=True, stop=True)
            m_sb = consts.tile([DA, DA], mm_dt, tag="m_sb")
            nc.vector.tensor_copy(m_sb[:], ps_m[:])

            wv_r = wv
            if USE_F32R:
                wv_r = consts.tile([DA, DA], F32R, tag="wv_r")
                nc.vector.tensor_copy(wv_r[:], wv[:])

            # ---- per-chunk accumulators ----
            accA = accp.tile([DA, CH], F32, tag="accA")
            accB = accp.tile([DA, CH], F32, tag="accB")
            nc.vector.memset(accA[:], 0.0)
            nc.vector.memset(accB[:], 0.0)

            # ---- slot loop ----
            for t in range(NSLOT):
                sl = slice(t * CH, (t + 1) * CH)
                xkv = slot_in.tile([DA, CH], F32, tag="xkv")
                xq = slot_in.tile([DA, CH], F32, tag="xq")
                nc.sync.dma_start(out=xkv[:], in_=xkvT_d[:, sl])
                nc.sync.dma_start(out=xq[:], in_=xqT_d[:, sl])
                if USE_F32R:
                    xkv_r = slot_in.tile([DA, CH], F32R, tag="xkv_r")
                    xq_r = slot_in.tile([DA, CH], F32R, tag="xq_r")
                    nc.vector.tensor_copy(xkv_r[:], xkv[:])
                    nc.vector.tensor_copy(xq_r[:], xq[:])
                else:
                    xkv_r, xq_r = xkv, xq

                # ktil = M^T @ xkvT
                ps_k = ps_k_p.tile([DA, CH], F32, tag="psk")
                nc.tensor.matmul(ps_k[:], m_sb[:], xkv_r[:], start=True, stop=True)
                kt = slot_mid.tile([DA, CH], mm_dt, tag="kt")
                nc.vector.tensor_copy(kt[:], ps_k[:])

                # v_aug per j-subchunk
                v_sbs = []
                for s in range(NJS):
                    ps_v = ps_v_p.tile([JS, DA], F32, tag="psv")
                    nc.tensor.matmul(
                        ps_v[:],
                        xkv_r[:, s * JS:(s + 1) * JS],
                        wv_r[:],
                        start=True,
                        stop=True,
                    )
                    v_sb = slot_mid.tile([JS, DA], mm_dt, tag=f"v{s}")
                    nc.vector.tensor_copy(v_sb[:], ps_v[:])
                    v_sbs.append(v_sb)

                # scores sT[j, i] per j-subchunk into one 4-bank PSUM tile
                ps_s = ps_s_p.tile([JS, NJS * CH], F32, tag="pss")
                for s in range(NJS):
                    nc.tensor.matmul(
                        ps_s[:, s * CH:(s + 1) * CH],
                        kt[:, s * JS:(s + 1) * JS],
                        xq_r[:],
                        start=True,
                        stop=True,
                    )

                # pT = exp(sT)
                pt = ptp.tile([JS, NJS * CH], mm_dt, tag="pt")
                nc.scalar.activation(
                    pt[:], ps_s[:], mybir.ActivationFunctionType.Exp
                )

                # causal mask on the two diagonal slots: keep j <= i
                if t < 2:
                    for s in range(NJS):
                        nc.gpsimd.affine_select(
                            out=pt[:, s * CH:(s + 1) * CH],
                            in_=pt[:, s * CH:(s + 1) * CH],
                            compare_op=mybir.AluOpType.is_ge,
                            fill=0.0,
                            base=-(s * JS),
                            pattern=[[1, CH]],
                            channel_multiplier=-1,
                        )

                # oT += v_aug^T @ pT  (row 64 accumulates the denominator)
                ps_o = ps_o_p.tile([DA, CH], F32, tag="pso")
                for s in range(NJS):
                    nc.tensor.matmul(
                        ps_o[:],
                        v_sbs[s][:],
                        pt[:, s * CH:(s + 1) * CH],
                        start=(s == 0),
                        stop=(s == NJS - 1),
                    )

                # acc{A,B} += gamma * partial, (1-gamma) * partial
                g = gdp.tile([DA, CH], F32, tag="g")
                d_ = gdp.tile([DA, CH], F32, tag="d")
                nc.vector.tensor_scalar_mul(g[:], ps_o[:], gam[:, t:t + 1])
                nc.vector.tensor_sub(d_[:], ps_o[:], g[:])
                nc.vector.tensor_add(accA[:], accA[:], g[:])
                nc.vector.tensor_add(accB[:], accB[:], d_[:])

            # ---- normalize + transpose back + store ----
            for pair, acc in enumerate((accA, accB)):
                for s in range(NJS):
                    ps_t = ps_v_p.tile([JS, DA], F32, tag="psv")
                    nc.tensor.transpose(
                        ps_t[:], acc[:, s * JS:(s + 1) * JS], ident[:]
                    )
                    r = finp.tile([JS, 1], F32, tag="r")
                    nc.vector.reciprocal(r[:], ps_t[:, D:DA])
                    o = finp.tile([JS, D], F32, tag="o")
                    nc.vector.tensor_scalar_mul(o[:], ps_t[:, 0:D], r[:])
                    nc.sync.dma_start(
                        out=out_d[pair, s * JS:(s + 1) * JS, :], in_=o[:]
                    )

    _split_multiwait(nc)
    return nc


_NC_CACHE = None


def _get_program():
    global _NC_CACHE
    if _NC_CACHE is None:
        _NC_CACHE = _build_program()
    return _NC_CACHE


def _host_inputs(x, w_q, b_q, w_k, b_k, w_v, b_v):
    """Per-core input dicts. Host work is layout only: transpose / gather /
    concat of x rows, weight reshuffles, and constant tables."""
    x = np.ascontiguousarray(np.asarray(x, dtype=np.float32))
    scale = 1.0 / np.sqrt(np.float32(D))

    wk_aug = np.concatenate([np.asarray(w_k, np.float32).T,
                             np.asarray(b_k, np.float32)[None, :]], axis=0)
    wq_aug = np.concatenate([np.asarray(w_q, np.float32).T,
                             np.asarray(b_q, np.float32)[None, :]], axis=0) * scale
    wv_aug = np.zeros((DA, DA), np.float32)
    wv_aug[:D, :D] = np.asarray(w_v, np.float32).T
    wv_aug[D, :D] = np.asarray(b_v, np.float32)
    wv_aug[D, D] = 1.0
    ident = np.eye(DA, dtype=np.float32)

    xT_aug = np.empty((DA, S), np.float32)
    xT_aug[:D] = x.T
    xT_aug[D] = 1.0

    in_maps = []
    for m in range(N_CORES):
        slots, gam = _schedule(m)
        xkvT = np.empty((DA, NSLOT * CH), np.float32)
        xqT = np.empty((DA, NSLOT * CH), np.float32)
        for t, (b, c) in enumerate(slots):
            xkvT[:, t * CH:(t + 1) * CH] = xT_aug[:, b * CH:(b + 1) * CH]
            xqT[:, t * CH:(t + 1) * CH] = xT_aug[:, c * CH:(c + 1) * CH]
        gam_np = np.broadcast_to(
            np.asarray(gam, np.float32)[None, :], (DA, NSLOT)
        ).copy()
        in_maps.append({
            "xkvT": xkvT,
            "xqT": xqT,
            "wkT": np.ascontiguousarray(wk_aug.T),
            "wqT": np.ascontiguousarray(wq_aug.T),
            "wv_aug": wv_aug,
            "gam": gam_np,
            "ident": ident,
        })
    return in_maps


def _assemble(results):
    out = np.empty((S, D), np.float32)
    for m in range(N_CORES):
        op = results[m]["out_pair"]
        A, B = m, NCH - 1 - m
        out[A * CH:(A + 1) * CH] = op[0]
        out[B * CH:(B + 1) * CH] = op[1]
    return out


def kernel(x, w_q, b_q, w_k, b_k, w_v, b_v):
    nc = _get_program()
    in_maps = _host_inputs(x, w_q, b_q, w_k, b_k, w_v, b_v)
    res = run_bass_kernel_spmd(nc, in_maps, list(range(N_CORES)))
    return _assemble(res.results)


# revision 5
# speedup vs baseline: 1.2745x; 1.2745x over previous
"""Causal single-head attention (S=8192, d=64) on 8 Trainium2 NeuronCores.

Strategy (sequence-parallel, load-balanced over the causal triangle):
  - Split the sequence into 16 chunks of 512 rows. Core m owns query chunks
    A=m and B=15-m, so every core sees (m+1) + (16-m) = 17 (kv-block, q-chunk)
    pairs of 512x512 scores -- perfectly balanced.
  - Each pair is one "slot". The SPMD program is identical on all cores; the
    per-core schedule is baked into the *input data* (host gathers the slot's
    kv rows and q rows, transposed and bias-augmented).
  - Scores are computed transposed, sT[j, i] (kv j on partitions), so the
    softmax denominator comes from an extra all-ones column of the augmented V
    in the PV matmul, and P^T feeds the PV matmul with no transposes at all.
  - The causal mask is needed only on the two diagonal slots (fixed slot
    positions 0 and 1) and is applied as a post-exp affine_select (zero the
    j > i half), which exactly matches softmax(scores - 1e10*mask).
  - Per-slot partials accumulate into per-chunk SBUF accumulators via a 0/1
    multiplier input (gamma), keeping the program fully static.

Math per slot t with kv block rows Xk = x[512b:512b+512], q rows Xq:
  xkvT = [Xk^T; 1]  (65 x 512)     xqT = [Xq^T; 1]  (65 x 512)
  M    = wk_aug @ wq_aug^T / 8     (65 x 65, on device from weight inputs)
  ktil = M^T @ xkvT                (65 x 512)
  sT   = ktil[:, js]^T @ xqT       (128 x 512 per 128-row j-subchunk, PSUM)
  pT   = exp(sT)                   (masked to causal on diagonal slots)
  oT  += v_aug[js]^T @ pT          (65 x 512; row 64 = softmax denominator)
Final: out[i, :] = oT[0:64, i] / oT[64, i], transposed back via PE transpose.
"""

import sys

sys.path.insert(0, "/opt/trn_rl_repo")

import numpy as np
import concourse.bass as bass
import concourse.mybir as mybir
from concourse import tile
from concourse.bass_utils import run_bass_kernel_spmd

N_CORES = 8
S = 8192
D = 64
CH = 512
NCH = S // CH          # 16 chunks
NSLOT = 17             # (kv, q) pairs per core
JS = 128               # j-subchunk (PSUM partition dim)
NJS = CH // JS         # 4
DA = D + 1             # bias-augmented contraction dim

F32 = mybir.dt.float32
F32R = mybir.dt.float32r

USE_F32R = True       # matmul operand precision: False = full fp32 (4 cyc/row)


def _split_multiwait(nc, max_waits=1):
    """The walrus build in this container accepts only one sync-wait per
    instruction; hoist extra waits onto preceding same-engine NOPs."""
    for func in nc.m.functions:
        for bb in func.blocks:
            new_insts = []
            for inst in bb.instructions:
                si = inst.sync_info
                if si is not None and si.on_wait and len(si.on_wait) > max_waits:
                    waits = list(si.on_wait)
                    rest, head = waits[:-max_waits], waits[-max_waits:]
                    for j, w in enumerate(rest):
                        nop = mybir.InstNoOp(
                            name=f"{inst.name}-wsplit{j}", ins=[], outs=[]
                        )
                        nop.engine = inst.engine
                        nop.sync_info = mybir.SyncInfo(on_wait=[w], on_update=[])
                        new_insts.append(nop)
                    inst.sync_info = mybir.SyncInfo(
                        on_wait=head, on_update=si.on_update
                    )
                new_insts.append(inst)
            bb.instructions = new_insts


def _schedule(m):
    """Slot list [(kv_block, q_chunk)] for core m; diagonal pairs first."""
    A, B = m, NCH - 1 - m
    slots = [(A, A), (B, B)]
    slots += [(b, A) for b in range(A)]
    slots += [(b, B) for b in range(B)]
    gam = [1.0, 0.0] + [1.0] * A + [0.0] * B
    assert len(slots) == NSLOT
    return slots, gam


def _build_program():
    nc = bass.Bass()
    mm_dt = F32R if USE_F32R else F32

    xkvT_d = nc.declare_dram_parameter("xkvT", [DA, NSLOT * CH], F32, isOutput=False)
    xqT_d = nc.declare_dram_parameter("xqT", [DA, NSLOT * CH], F32, isOutput=False)
    wkT_d = nc.declare_dram_parameter("wkT", [D, DA], F32, isOutput=False)
    wqT_d = nc.declare_dram_parameter("wqT", [D, DA], F32, isOutput=False)
    wv_d = nc.declare_dram_parameter("wv_aug", [DA, DA], F32, isOutput=False)
    gam_d = nc.declare_dram_parameter("gam", [DA, NSLOT], F32, isOutput=False)
    ident_d = nc.declare_dram_parameter("ident", [DA, DA], F32, isOutput=False)
    out_d = nc.declare_dram_parameter("out_pair", [2, CH, D], F32, isOutput=True)

    with tile.TileContext(nc) as tc:
        with (
            tc.tile_pool(name="consts", bufs=1) as consts,
            tc.tile_pool(name="acc", bufs=1) as accp,
            tc.tile_pool(name="slot_in", bufs=3) as slot_in,
            tc.tile_pool(name="slot_mid", bufs=3) as slot_mid,
            tc.tile_pool(name="pt", bufs=2) as ptp,
            tc.tile_pool(name="gd", bufs=3) as gdp,
            tc.tile_pool(name="fin", bufs=2) as finp,
            tc.tile_pool(name="ps_s", bufs=1, space="PSUM") as ps_s_p,
            tc.tile_pool(name="ps_o", bufs=2, space="PSUM") as ps_o_p,
            tc.tile_pool(name="ps_k", bufs=1, space="PSUM") as ps_k_p,
            tc.tile_pool(name="ps_v", bufs=1, space="PSUM") as ps_v_p,
        ):
            # ---- constants ----
            wkT = consts.tile([D, DA], F32)
            wqT = consts.tile([D, DA], F32)
            wv = consts.tile([DA, DA], F32)
            gam = consts.tile([DA, NSLOT], F32)
            ident = consts.tile([DA, DA], F32)
            nc.sync.dma_start(out=wkT[:], in_=wkT_d[:])
            nc.sync.dma_start(out=wqT[:], in_=wqT_d[:])
            nc.sync.dma_start(out=wv[:], in_=wv_d[:])
            nc.sync.dma_start(out=gam[:], in_=gam_d[:])
            nc.sync.dma_start(out=ident[:], in_=ident_d[:])

            # M = wk_aug @ wq_aug^T  (scale folded into wq on host)
            ps_m = ps_k_p.tile([DA, DA], F32, tag="psk")
            nc.tensor.matmul(ps_m[:], wkT[:], wqT[:], start=True, stop=True)
            m_sb = consts.tile([DA, DA], mm_dt, tag="m_sb")
            nc.vector.tensor_copy(m_sb[:], ps_m[:])

            # v matmuls stay fp32: their moving dim is 65 (odd, fp32r ISA
            # rejects it) and N<256 gets no fp32r speedup anyway.
            wv_r = wv

            # ---- per-chunk accumulators ----
            accA = accp.tile([DA, CH], F32, tag="accA")
            accB = accp.tile([DA, CH], F32, tag="accB")
            nc.vector.memset(accA[:], 0.0)
            nc.vector.memset(accB[:], 0.0)

            # ---- slot loop ----
            for t in range(NSLOT):
                sl = slice(t * CH, (t + 1) * CH)
                xkv = slot_in.tile([DA, CH], F32, tag="xkv")
                xq = slot_in.tile([DA, CH], F32, tag="xq")
                nc.sync.dma_start(out=xkv[:], in_=xkvT_d[:, sl])
                nc.sync.dma_start(out=xq[:], in_=xqT_d[:, sl])
                if USE_F32R:
                    xkv_r = slot_in.tile([DA, CH], F32R, tag="xkv_r")
                    xq_r = slot_in.tile([DA, CH], F32R, tag="xq_r")
                    nc.vector.tensor_copy(xkv_r[:], xkv[:])
                    nc.vector.tensor_copy(xq_r[:], xq[:])
                else:
                    xkv_r, xq_r = xkv, xq

                # ktil = M^T @ xkvT
                ps_k = ps_k_p.tile([DA, CH], F32, tag="psk")
                nc.tensor.matmul(ps_k[:], m_sb[:], xkv_r[:], start=True, stop=True)
                kt = slot_mid.tile([DA, CH], mm_dt, tag="kt")
                nc.vector.tensor_copy(kt[:], ps_k[:])

                # v_aug per j-subchunk
                v_sbs = []
                for s in range(NJS):
                    ps_v = ps_v_p.tile([JS, DA], F32, tag="psv")
                    nc.tensor.matmul(
                        ps_v[:],
                        xkv[:, s * JS:(s + 1) * JS],
                        wv_r[:],
                        start=True,
                        stop=True,
                    )
                    v_sb = slot_mid.tile([JS, DA], mm_dt, tag=f"v{s}")
                    nc.vector.tensor_copy(v_sb[:], ps_v[:])
                    v_sbs.append(v_sb)

                # scores sT[j, i] per j-subchunk into one 4-bank PSUM tile
                ps_s = ps_s_p.tile([JS, NJS * CH], F32, tag="pss")
                for s in range(NJS):
                    nc.tensor.matmul(
                        ps_s[:, s * CH:(s + 1) * CH],
                        kt[:, s * JS:(s + 1) * JS],
                        xq_r[:],
                        start=True,
                        stop=True,
                    )

                # pT = exp(sT)
                pt = ptp.tile([JS, NJS * CH], mm_dt, tag="pt")
                nc.scalar.activation(
                    pt[:], ps_s[:], mybir.ActivationFunctionType.Exp
                )

                # causal mask on the two diagonal slots: keep j <= i
                if t < 2:
                    for s in range(NJS):
                        nc.gpsimd.affine_select(
                            out=pt[:, s * CH:(s + 1) * CH],
                            in_=pt[:, s * CH:(s + 1) * CH],
                            compare_op=mybir.AluOpType.is_ge,
                            fill=0.0,
                            base=-(s * JS),
                            pattern=[[1, CH]],
                            channel_multiplier=-1,
                        )

                # oT += v_aug^T @ pT  (row 64 accumulates the denominator)
                ps_o = ps_o_p.tile([DA, CH], F32, tag="pso")
                for s in range(NJS):
                    nc.tensor.matmul(
                        ps_o[:],
                        v_sbs[s][:],
                        pt[:, s * CH:(s + 1) * CH],
                        start=(s == 0),
                        stop=(s == NJS - 1),
                    )

                # acc{A,B} += gamma * partial, (1-gamma) * partial
                g = gdp.tile([DA, CH], F32, tag="g")
                d_ = gdp.tile([DA, CH], F32, tag="d")
                nc.vector.tensor_scalar_mul(g[:], ps_o[:], gam[:, t:t + 1])
                nc.vector.tensor_sub(d_[:], ps_o[:], g[:])
                nc.vector.tensor_add(accA[:], accA[:], g[:])
                nc.vector.tensor_add(accB[:], accB[:], d_[:])

            # ---- normalize + transpose back + store ----
            for pair, acc in enumerate((accA, accB)):
                for s in range(NJS):
                    ps_t = ps_v_p.tile([JS, DA], F32, tag="psv")
                    nc.tensor.transpose(
                        ps_t[:], acc[:, s * JS:(s + 1) * JS], ident[:]
                    )
                    r = finp.tile([JS, 1], F32, tag="r")
                    nc.vector.reciprocal(r[:], ps_t[:, D:DA])
                    o = finp.tile([JS, D], F32, tag="o")
                    nc.vector.tensor_scalar_mul(o[:], ps_t[:, 0:D], r[:])
                    nc.sync.dma_start(
                        out=out_d[pair, s * JS:(s + 1) * JS, :], in_=o[:]
                    )

    _split_multiwait(nc)
    return nc


_NC_CACHE = None


def _get_program():
    global _NC_CACHE
    if _NC_CACHE is None:
        _NC_CACHE = _build_program()
    return _NC_CACHE


def _host_inputs(x, w_q, b_q, w_k, b_k, w_v, b_v):
    """Per-core input dicts. Host work is layout only: transpose / gather /
    concat of x rows, weight reshuffles, and constant tables."""
    x = np.ascontiguousarray(np.asarray(x, dtype=np.float32))
    scale = 1.0 / np.sqrt(np.float32(D))

    wk_aug = np.concatenate([np.asarray(w_k, np.float32).T,
                             np.asarray(b_k, np.float32)[None, :]], axis=0)
    wq_aug = np.concatenate([np.asarray(w_q, np.float32).T,
                             np.asarray(b_q, np.float32)[None, :]], axis=0) * scale
    wv_aug = np.zeros((DA, DA), np.float32)
    wv_aug[:D, :D] = np.asarray(w_v, np.float32).T
    wv_aug[D, :D] = np.asarray(b_v, np.float32)
    wv_aug[D, D] = 1.0
    ident = np.eye(DA, dtype=np.float32)

    xT_aug = np.empty((DA, S), np.float32)
    xT_aug[:D] = x.T
    xT_aug[D] = 1.0

    in_maps = []
    for m in range(N_CORES):
        slots, gam = _schedule(m)
        xkvT = np.empty((DA, NSLOT * CH), np.float32)
        xqT = np.empty((DA, NSLOT * CH), np.float32)
        for t, (b, c) in enumerate(slots):
            xkvT[:, t * CH:(t + 1) * CH] = xT_aug[:, b * CH:(b + 1) * CH]
            xqT[:, t * CH:(t + 1) * CH] = xT_aug[:, c * CH:(c + 1) * CH]
        gam_np = np.broadcast_to(
            np.asarray(gam, np.float32)[None, :], (DA, NSLOT)
        ).copy()
        in_maps.append({
            "xkvT": xkvT,
            "xqT": xqT,
            "wkT": np.ascontiguousarray(wk_aug.T),
            "wqT": np.ascontiguousarray(wq_aug.T),
            "wv_aug": wv_aug,
            "gam": gam_np,
            "ident": ident,
        })
    return in_maps


def _assemble(results):
    out = np.empty((S, D), np.float32)
    for m in range(N_CORES):
        op = results[m]["out_pair"]
        A, B = m, NCH - 1 - m
        out[A * CH:(A + 1) * CH] = op[0]
        out[B * CH:(B + 1) * CH] = op[1]
    return out


def kernel(x, w_q, b_q, w_k, b_k, w_v, b_v):
    nc = _get_program()
    in_maps = _host_inputs(x, w_q, b_q, w_k, b_k, w_v, b_v)
    res = run_bass_kernel_spmd(nc, in_maps, list(range(N_CORES)))
    return _assemble(res.results)
